# revision 10
# baseline (speedup 1.0000x reference)
"""Trainium2 Bass kernel for nn_EnsembleClustering_62646392979777.

Strategy (validated against the reference by a numpy prototype):
  * The full-resolution projection y = proj(x) is only ever consumed through
    spatial average-pools (7x7 agents, 2x2 clusters), and pooling commutes
    with the 1x1 conv.  So pool x first (56x56 -> 14x14 -> {7x7, 2x2}),
    then project the 53 pooled positions instead of 3136.  This removes
    ~98% of the FLOPs; the kernel becomes HBM-bound on reading x and
    writing the output.
  * proj2 and the bilinear upsample also commute: run proj2 on the 7x7
    grid, then upsample as a dense [49 x 3136] matmul (exact linear op).
  * Data-parallel over batch: 16 batches -> 8 cores x 2.
  * The 8 cores share one chip's HBM, so bytes are the roofline: x and y
    travel as bf16 (host casts, ~0.5% l2 error, gate is 2e-2).  Matmuls
    run bf16 (PE 1 cycle/row vs 4 for fp32); softmax/norm math stays fp32.
  * Pooling is restructured as packed bf16 tensor_tensor adds (DVE 2x
    mode) for the h-direction plus strided pair-adds for w, instead of
    monolithic reduces (reduces get no DVE fast modes).
  * All scalar-engine activations use one table set (exp/ln/square/copy)
    to avoid 1.3us act-table reloads; 1/sqrt is exp(-0.5*ln(s)).
  * Softmax max-subtraction is dropped: pooled scores are O(0.03), and
    |similarity| <= |alpha|+|beta| ~ 1, so exp never overflows fp32.

Layout rules honored throughout: compute-engine partition bases are
32-aligned (BIR-verifier enforced); matmul operand bases in {0,32,64};
per-slice work is stacked along the free dimension.
"""
import sys
import numpy as np

sys.path.insert(0, "/opt/trn_rl_repo")

import ml_dtypes  # noqa: E402
import concourse.bass as bass  # noqa: E402
import concourse.tile as tile  # noqa: E402
from concourse import bacc, mybir  # noqa: E402
from concourse.bass_utils import run_bass_kernel_spmd  # noqa: E402
from concourse.masks import make_identity  # noqa: E402

F32 = mybir.dt.float32
BF16 = mybir.dt.bfloat16
AX = mybir.AxisListType
AF = mybir.ActivationFunctionType
OP = mybir.AluOpType

EPS = 1e-6
INV_SQRT_C = float(1.0 / np.sqrt(np.float32(48.0)))

_CACHE = {}


def _upsample_matrix():
    # jax.image.resize 'linear' 7->56 upsample: half-pixel centers, edge clamp
    U = np.zeros((56, 7), dtype=np.float64)
    for o in range(56):
        src = (o + 0.5) / 8.0 - 0.5
        i0 = int(np.floor(src))
        t = src - i0
        U[o, min(max(i0, 0), 6)] += 1.0 - t
        U[o, min(max(i0 + 1, 0), 6)] += t
    U = U.astype(np.float32)
    return np.einsum("Oi,Pj->ijOP", U, U).reshape(49, 3136).copy()


def build_nc(reps=1, stage="full", loop=False):
    # stage: "full" | "pool" (DMA-in + pooling only) | "noup" (skip upsample)
    # loop=True wraps the body in a hardware For_i loop (reps iterations of
    # a 2-rep body) so steady-state timing NEFFs compile in O(1) and run
    # long enough to swamp dispatch jitter.
    nc = bacc.Bacc("TRN2", target_bir_lowering=False, debug=False,
                   enable_asserts=False)

    x_d = nc.dram_tensor("x", [2, 384, 3136], BF16, kind="ExternalInput").ap()
    pwt_d = nc.dram_tensor("pwt", [128, 3, 1920], BF16, kind="ExternalInput").ap()
    pbn_d = nc.dram_tensor("pbn", [48, 24], F32, kind="ExternalInput").ap()
    pbv_d = nc.dram_tensor("pbv", [1, 768], F32, kind="ExternalInput").ap()
    p2b_d = nc.dram_tensor("p2b", [1, 384], F32, kind="ExternalInput").ap()
    al_d = nc.dram_tensor("alph", [1, 64], F32, kind="ExternalInput").ap()
    be_d = nc.dram_tensor("beta", [1, 64], F32, kind="ExternalInput").ap()
    p2w_d = nc.dram_tensor("p2w", [48, 8, 384], BF16, kind="ExternalInput").ap()
    mup_d = nc.dram_tensor("mup", [49, 3136], BF16, kind="ExternalInput").ap()
    y_d = nc.dram_tensor("y", [2, 384, 3136], BF16, kind="ExternalOutput").ap()

    with tile.TileContext(nc) as tc:
        with tc.tile_pool(name="w", bufs=1) as wp, \
             tc.tile_pool(name="xin", bufs=3) as xin, \
             tc.tile_pool(name="pool", bufs=2) as pp, \
             tc.tile_pool(name="st", bufs=1) as st, \
             tc.tile_pool(name="out", bufs=2) as outp, \
             tc.tile_pool(name="ps", bufs=2, space="PSUM") as ps:

            # ---------------- constants & weights ----------------
            identB = wp.tile([128, 128], BF16, tag="identB")
            make_identity(nc, identB[:])
            ones_c = wp.tile([49, 1], F32, tag="ones_c")
            nc.vector.memset(ones_c[:], 1.0)
            ones_cb = wp.tile([49, 1], BF16, tag="ones_cb")
            nc.vector.memset(ones_cb[:], 1.0)
            ones_r = wp.tile([1, 768], F32, tag="ones_r")
            nc.vector.memset(ones_r[:], 1.0)

            PWT = wp.tile([128, 3, 1920], BF16, tag="pwt")
            nc.sync.dma_start(PWT[:], pwt_d)
            P2W = wp.tile([48, 8, 384], BF16, tag="p2w")
            nc.sync.dma_start(P2W[:], p2w_d)
            MUP = wp.tile([49, 3136], BF16, tag="mup")
            nc.sync.dma_start(MUP[:], mup_d)
            PBN = wp.tile([48, 24], F32, tag="pbn")
            nc.sync.dma_start(PBN[:], pbn_d)
            PBV1 = wp.tile([1, 768], F32, tag="pbv1")
            nc.sync.dma_start(PBV1[:], pbv_d)
            P2B1 = wp.tile([1, 384], F32, tag="p2b1")
            nc.sync.dma_start(P2B1[:], p2b_d)
            AL1 = wp.tile([1, 64], F32, tag="al1")
            nc.sync.dma_start(AL1[:], al_d)
            BE1 = wp.tile([1, 64], F32, tag="be1")
            nc.sync.dma_start(BE1[:], be_d)

            # broadcast alpha/beta/bias rows across partitions via K=1 matmuls
            ALB = wp.tile([49, 64], F32, tag="alb")
            BEB = wp.tile([49, 64], F32, tag="beb")
            for src, dst in ((AL1, ALB), (BE1, BEB)):
                pt = ps.tile([49, 64], F32, tag="C")
                nc.tensor.matmul(pt[:], ones_r[:, :49], src[:], start=True, stop=True)
                nc.vector.tensor_copy(dst[:], pt[:])
            PB49 = wp.tile([49, 768], F32, tag="pb49")
            PB4 = wp.tile([4, 768], F32, tag="pb4")
            for half in range(2):
                pt = ps.tile([49, 384], F32, tag="B")
                nc.tensor.matmul(pt[:], ones_r[:, :49], PBV1[:, 384 * half:384 * (half + 1)],
                                 start=True, stop=True)
                nc.vector.tensor_copy(PB49[:, 384 * half:384 * (half + 1)], pt[:])
                pt2 = ps.tile([4, 384], F32, tag="C")
                nc.tensor.matmul(pt2[:], ones_r[:, :4], PBV1[:, 384 * half:384 * (half + 1)],
                                 start=True, stop=True)
                nc.vector.tensor_copy(PB4[:, 384 * half:384 * (half + 1)], pt2[:])
            PB2T = wp.tile([49, 384], F32, tag="pb2t")
            pt = ps.tile([49, 384], F32, tag="B")
            nc.tensor.matmul(pt[:], ones_r[:, :49], P2B1[:], start=True, stop=True)
            nc.vector.tensor_copy(PB2T[:], pt[:])

            # ---------------- per-batch pipeline ----------------
            # reps>1 re-emits the body for steady-state benchmarking
            def _body(n_reps):
              for rep_bi in range(2 * n_reps):
                bi = rep_bi % 2
                # ---- Stage A: load & pool x (bf16) ----
                # h-pools as packed pair-adds (DVE 2x mode), w-pools strided.
                XP = st.tile([128, 3, 53], BF16, tag=f"xp{bi}")
                for j in range(3):
                    X = xin.tile([128, 3136], BF16, tag="x")
                    nc.sync.dma_start(X[:], x_d[bi, 128 * j:128 * (j + 1), :])
                    Xv = X[:].rearrange("p (a two b) -> p a two b", two=2, b=56)
                    A = pp.tile([128, 28, 56], BF16, tag="pa")
                    nc.vector.tensor_tensor(A[:], Xv[:, :, 0, :], Xv[:, :, 1, :], OP.add)
                    Av = A[:].rearrange("p (a two) b -> p a two b", two=2)
                    Bt = pp.tile([128, 14, 56], BF16, tag="pb")
                    nc.vector.tensor_tensor(Bt[:], Av[:, :, 0, :], Av[:, :, 1, :], OP.add)
                    Bv = Bt[:].rearrange("p a (c two) -> p a c two", two=2)
                    # w-pools on gpsimd (SBUF-only bf16): frees the DVE for
                    # the PSUM->SBUF copies it alone (with ACT) can do.
                    D = pp.tile([128, 14, 28], BF16, tag="pd")
                    nc.gpsimd.tensor_tensor(D[:], Bv[:, :, :, 0], Bv[:, :, :, 1], OP.add)
                    Dv = D[:].rearrange("p a (c two) -> p a c two", two=2)
                    C14 = pp.tile([128, 14, 14], BF16, tag="pc")
                    nc.gpsimd.tensor_tensor(C14[:], Dv[:, :, :, 0], Dv[:, :, :, 1], OP.add)
                    Cv = C14[:].rearrange("p (g two) w -> p g two w", two=2)
                    E = pp.tile([128, 7, 14], BF16, tag="pe")
                    nc.vector.tensor_tensor(E[:], Cv[:, :, 0, :], Cv[:, :, 1, :], OP.add)
                    Ev = E[:].rearrange("p g (f two) -> p g f two", two=2)
                    F = pp.tile([128, 7, 7], BF16, tag="pf")
                    nc.vector.tensor_tensor(F[:], Ev[:, :, :, 0], Ev[:, :, :, 1], OP.add)
                    nc.vector.tensor_scalar_mul(
                        XP[:, j, 0:49], F[:].rearrange("p a b -> p (a b)"), 1.0 / 64.0)
                    P2t = pp.tile([128, 2, 2], F32, tag="pg")
                    nc.vector.reduce_sum(
                        P2t[:],
                        C14[:].rearrange("p (oh hi) (ow wi) -> p oh ow hi wi",
                                         oh=2, ow=2),
                        axis=AX.XY)
                    nc.vector.tensor_scalar_mul(
                        XP[:, j, 49:53], P2t[:].rearrange("p a b -> p (a b)"),
                        1.0 / 784.0)

                if stage == "pool":
                    ri = (rep_bi // 2) % 19
                    nc.sync.dma_start(
                        y_d[bi, 0:128, 159 * ri:159 * (ri + 1)],
                        XP[:].rearrange("p a b -> p (a b)"))
                    continue

                # ---- Stage B1: natural-layout projection (groups p, k0, k1) ----
                # chunk t = g*8 + h ; o-range = G[g]*384 + 48h,  G = [0, 1, 3]
                # k-groups (g=1,2) first: the score matmuls depend only on them.
                Ysb = st.tile([48, 24, 53], BF16, tag=f"ysb{bi}")
                for gi, (g, G) in enumerate(((1, 1), (2, 3), (0, 0))):
                    pt = ps.tile([48, 8, 53], F32, tag="A")
                    for h in range(8):
                        o0 = G * 384 + 48 * h
                        for j in range(3):
                            nc.tensor.matmul(pt[:, h, :], PWT[:, j, o0:o0 + 48],
                                             XP[:, j, :], start=(j == 0), stop=(j == 2))
                    nc.vector.tensor_tensor(
                        Ysb[:, 8 * g:8 * (g + 1), :], pt[:],
                        PBN[:, 8 * g:8 * (g + 1), None].to_broadcast((48, 8, 53)),
                        OP.add)

                # ---- Stage B2: transposed projection of v-groups ----
                # V7T[n, i*384 + hc] = y7[(2+2i)*384 + hc, n];  VCt likewise
                V7T = st.tile([49, 768], BF16, tag=f"v7t{bi}")
                VCt = st.tile([4, 768], BF16, tag=f"vct{bi}")
                for i in range(2):
                    o0 = (2 + 2 * i) * 384
                    pt = ps.tile([49, 384], F32, tag="B")
                    for j in range(3):
                        nc.tensor.matmul(pt[:], XP[:, j, 0:49], PWT[:, j, o0:o0 + 384],
                                         start=(j == 0), stop=(j == 2))
                    nc.vector.tensor_tensor(V7T[:, 384 * i:384 * (i + 1)], pt[:],
                                            PB49[:, 384 * i:384 * (i + 1)], OP.add)
                    pt2 = ps.tile([4, 384], F32, tag="C")
                    for j in range(3):
                        nc.tensor.matmul(pt2[:], XP[:, j, 49:53], PWT[:, j, o0:o0 + 384],
                                         start=(j == 0), stop=(j == 2))
                    nc.vector.tensor_tensor(VCt[:, 384 * i:384 * (i + 1)], pt2[:],
                                            PB4[:, 384 * i:384 * (i + 1)], OP.add)

                # p-token norms early: independent of the attention chain
                # rp = (sum_c p^2)^-1/2 via exp(-0.5 * ln s): stays in one
                # activation-table set (set 6: exp/ln/square/copy).
                SQP = st.tile([48, 8, 49], F32, tag=f"sqp{bi}")
                nc.scalar.activation(SQP[:], Ysb[:, 0:8, 0:49], AF.Square)
                ptn = ps.tile([49, 8], F32, tag="C")
                for h in range(8):
                    nc.tensor.matmul(ptn[:, h:h + 1], SQP[:, h, :], ones_c[:48, :],
                                     start=True, stop=True)
                RP = st.tile([49, 8], F32, tag=f"rp{bi}")
                nc.scalar.activation(RP[:], ptn[:], AF.Ln)
                nc.scalar.activation(RP[:], RP[:], AF.Exp, scale=-0.5)

                # ---- Stage C: attention / clustering (8 slices, free-stacked) ----
                # module-0 scores -> all 8 slices in one PSUM bank; exp reads
                # PSUM directly with the 1/sqrt(c) scale folded in (no max
                # subtraction needed: |scores| ~ 0.03).
                pt0 = ps.tile([4, 8, 49], F32, tag="C")
                pt1 = ps.tile([49, 8, 4], F32, tag="B")
                for h in range(8):
                    nc.tensor.matmul(pt0[:, h, :], Ysb[:, 8 + h, 49:53],
                                     Ysb[:, 8 + h, 0:49], start=True, stop=True)
                    nc.tensor.matmul(pt1[:, h, :], Ysb[:, 16 + h, 0:49],
                                     Ysb[:, 16 + h, 49:53], start=True, stop=True)

                # softmax0 over n (module 0)
                E0 = st.tile([4, 8, 49], F32, tag=f"e0{bi}")
                nc.scalar.activation(E0[:], pt0[:], AF.Exp, scale=INV_SQRT_C)
                SM0 = st.tile([4, 8], F32, tag=f"sm0{bi}")
                nc.vector.reduce_sum(SM0[:], E0[:], axis=AX.X)
                nc.vector.reciprocal(SM0[:], SM0[:])
                A0 = st.tile([4, 8, 49], BF16, tag=f"a0{bi}")
                nc.vector.tensor_tensor(A0[:], E0[:],
                                        SM0[:, :, None].to_broadcast((4, 8, 49)), OP.mult)

                # softmax1 over clusters (module 1)
                E1 = st.tile([49, 8, 4], F32, tag=f"e1{bi}")
                nc.scalar.activation(E1[:], pt1[:], AF.Exp, scale=INV_SQRT_C)
                SM1 = st.tile([49, 8], F32, tag=f"sm1{bi}")
                nc.vector.reduce_sum(SM1[:], E1[:], axis=AX.X)
                nc.vector.reciprocal(SM1[:], SM1[:])
                A1T = st.tile([49, 8, 4], BF16, tag=f"a1t{bi}")
                nc.vector.tensor_tensor(A1T[:], E1[:],
                                        SM1[:, :, None].to_broadcast((49, 8, 4)), OP.mult)

                # transpose module-0 attention: A0 [4,49] slices -> A0T [49,(8,4)]
                ptA = ps.tile([49, 8, 4], BF16, tag="C")
                for h in range(8):
                    nc.tensor.transpose(ptA[:, h, :], A0[:, h, :], identB[:4, :4])
                A0T = st.tile([49, 8, 4], BF16, tag=f"a0t{bi}")
                nc.vector.tensor_copy(A0T[:], ptA[:])

                # fuzzy-membership normalizer: 1/(sum_n memb + eps), PE-broadcast
                ptd = ps.tile([1, 32], F32, tag="C")
                nc.tensor.matmul(ptd[:], ones_cb[:], A1T[:].rearrange("p a b -> p (a b)"),
                                 start=True, stop=True)
                DE = st.tile([1, 32], F32, tag=f"de{bi}")
                nc.vector.tensor_scalar_add(DE[:], ptd[:], EPS)
                nc.vector.reciprocal(DE[:], DE[:])
                ptb = ps.tile([49, 32], F32, tag="B")
                nc.tensor.matmul(ptb[:], ones_r[:, :49], DE[:], start=True, stop=True)
                A1N = st.tile([49, 8, 4], BF16, tag=f"a1n{bi}")
                nc.vector.tensor_tensor(A1N[:].rearrange("p a b -> p (a b)"),
                                        A1T[:].rearrange("p a b -> p (a b)"),
                                        ptb[:], OP.mult)

                # agg = attn @ v (+ vc), stacked [4, (2, 8, 48)]
                AGGf = st.tile([4, 2, 8, 48], BF16, tag=f"aggf{bi}")
                for i in range(2):
                    AT = A0T if i == 0 else A1N
                    ptg = ps.tile([4, 8, 48], F32, tag="C" if i == 0 else "B")
                    for h in range(8):
                        nc.tensor.matmul(ptg[:, h, :], AT[:, h, :],
                                         V7T[:, 384 * i + 48 * h:384 * i + 48 * (h + 1)],
                                         start=True, stop=True)
                    nc.vector.tensor_tensor(AGGf[:, i, :, :], ptg[:],
                                      VCt[:, 384 * i:384 * (i + 1)].rearrange(
                                          "p (a b) -> p a b", b=48), OP.add)

                # agg row norms -> normalized AGGN (rsqrt via exp(-0.5 ln))
                SQ = st.tile([4, 768], F32, tag=f"sq{bi}")
                nc.scalar.activation(SQ[:], AGGf[:].rearrange("p a b c -> p (a b c)"),
                                     AF.Square)
                SS = st.tile([4, 16], F32, tag=f"ss{bi}")
                nc.vector.reduce_sum(SS[:], SQ[:].rearrange("p (g c) -> p g c", c=48),
                                     axis=AX.X)
                nc.scalar.activation(SS[:], SS[:], AF.Ln)
                nc.scalar.activation(SS[:], SS[:], AF.Exp, scale=-0.5)
                AGGN = st.tile([4, 2, 8, 48], BF16, tag=f"aggn{bi}")
                nc.vector.tensor_tensor(
                    AGGN[:].rearrange("p a b c -> p (a b) c"),
                    AGGf[:].rearrange("p a b c -> p (a b) c"),
                    SS[:, :, None].to_broadcast((4, 16, 48)), OP.mult)

                # transpose AGGN slices -> AGGNT [48, (8 slices, 8 m)]
                ptT = ps.tile([48, 8, 8], BF16, tag="C")
                for i in range(2):
                    for h in range(8):
                        nc.tensor.transpose(ptT[:, h, 4 * i:4 * (i + 1)],
                                            AGGN[:, i, h, :], identB[:4, :4])
                AGGNT = st.tile([48, 8, 8], BF16, tag=f"aggnt{bi}")
                nc.scalar.copy(AGGNT[:], ptT[:])

                # sim^T [49, (8 slices, 8 m)] = (p^T @ aggn^T) * rp, alpha/beta
                ptm = ps.tile([49, 64], F32, tag="C")
                for h in range(8):
                    nc.tensor.matmul(ptm[:, 8 * h:8 * (h + 1)], Ysb[:, h, 0:49],
                                     AGGNT[:, h, :], start=True, stop=True)
                SIMT = st.tile([49, 8, 8], F32, tag=f"simt{bi}")
                nc.vector.tensor_tensor(SIMT[:], ptm[:].rearrange("p (a b) -> p a b", b=8),
                                        RP[:, :, None].to_broadcast((49, 8, 8)), OP.mult)
                nc.vector.tensor_tensor(SIMT[:].rearrange("p a b -> p (a b)"),
                                        SIMT[:].rearrange("p a b -> p (a b)"),
                                        ALB[:], OP.mult)
                nc.vector.tensor_tensor(SIMT[:].rearrange("p a b -> p (a b)"),
                                        SIMT[:].rearrange("p a b -> p (a b)"),
                                        BEB[:], OP.add)

                # assignment softmax over the 8 clusters (inner free dim);
                # |similarity| <= 1 so no max subtraction.
                EX = st.tile([49, 8, 8], F32, tag=f"ex{bi}")
                nc.scalar.activation(EX[:], SIMT[:], AF.Exp)
                SMS = st.tile([49, 8], F32, tag=f"sms{bi}")
                nc.vector.reduce_sum(SMS[:], EX[:], axis=AX.X)
                nc.vector.reciprocal(SMS[:], SMS[:])
                ASGT = st.tile([49, 8, 8], BF16, tag=f"asgt{bi}")
                nc.vector.tensor_tensor(ASGT[:], EX[:],
                                        SMS[:, :, None].to_broadcast((49, 8, 8)), OP.mult)

                # transpose assignment to m-on-partitions (two 4-row banks)
                # inner dim padded to 50: bf16 PSUM sub-writes must stay
                # 4-byte aligned (49*2 = 98B offsets are not)
                ptS0 = ps.tile([4, 8, 50], BF16, tag="C")
                ptS1 = ps.tile([4, 8, 50], BF16, tag="B")
                for h in range(8):
                    nc.tensor.transpose(ptS0[:, h, 0:49], ASGT[:, h, 0:4], identB[:49, :49])
                    nc.tensor.transpose(ptS1[:, h, 0:49], ASGT[:, h, 4:8], identB[:49, :49])
                ASG0 = st.tile([4, 8, 49], BF16, tag=f"asg0{bi}")
                ASG1 = st.tile([4, 8, 49], BF16, tag=f"asg1{bi}")
                nc.scalar.copy(ASG0[:], ptS0[:, :, 0:49])
                nc.vector.tensor_copy(ASG1[:], ptS1[:, :, 0:49])

                # out_low per slice: z [48, 49] = agg^T @ assignment
                ptZ = ps.tile([48, 8, 49], F32, tag="A")
                for h in range(8):
                    nc.tensor.matmul(ptZ[:, h, :], AGGf[:, 0, h, :], ASG0[:, h, :],
                                     start=True, stop=False)
                    nc.tensor.matmul(ptZ[:, h, :], AGGf[:, 1, h, :], ASG1[:, h, :],
                                     start=False, stop=True)
                Zf = st.tile([48, 8, 49], BF16, tag=f"zf{bi}")
                nc.scalar.copy(Zf[:], ptZ[:])

                # proj2 on the 7x7 grid, transposed layout: [49, 384] out
                ptP = ps.tile([49, 384], F32, tag="B")
                for h in range(8):
                    nc.tensor.matmul(ptP[:], Zf[:, h, :], P2W[:, h, :],
                                     start=(h == 0), stop=(h == 7))
                Z2T = st.tile([49, 384], BF16, tag=f"z2t{bi}")
                nc.vector.tensor_tensor(Z2T[:], ptP[:], PB2T[:], OP.add)

                if stage == "noup":
                    ri = (rep_bi // 2) % 8
                    nc.sync.dma_start(y_d[bi, 0:49, 384 * ri:384 * (ri + 1)], Z2T[:])
                    continue

                # upsample: out[128, 3136] = z2T^T @ MUP, tile N by 448
                for oi in range(3):
                    OUT = outp.tile([128, 3136], BF16, tag="out")
                    for nt in range(7):
                        pt = ps.tile([128, 448], F32, tag="U")
                        nc.tensor.matmul(pt[:],
                                         Z2T[:, 128 * oi:128 * (oi + 1)],
                                         MUP[:, 448 * nt:448 * (nt + 1)],
                                         start=True, stop=True)
                        eng = nc.vector if nt % 2 == 1 else nc.scalar
                        if eng is nc.vector:
                            nc.vector.tensor_copy(OUT[:, 448 * nt:448 * (nt + 1)], pt[:])
                        else:
                            nc.scalar.copy(OUT[:, 448 * nt:448 * (nt + 1)], pt[:])
                    nc.sync.dma_start(y_d[bi, 128 * oi:128 * (oi + 1), :], OUT[:])

            if loop:
                assert reps % 2 == 0
                with tc.For_i(0, reps // 2, 1,
                              hint_engines=(mybir.EngineType.PE,)):
                    _body(2)
            else:
                _body(reps)

    nc.compile()
    _fix_act_table_loads(nc)
    return nc


def _fix_act_table_loads(nc):
    # Every activation used here (exp/ln/square/copy/identity) lives in
    # act-table set 6 (natural_log_exp_and_others), but the auto-inserted
    # loads pick the first set per function (5 for ln, 0 for exp), so the
    # 1.3us table load thrashes before nearly every Ln/Exp pair.  Rewrite
    # to a single set-6 load per block.
    for blk in nc.m.functions[0].blocks:
        idxs = [i for i, inst in enumerate(blk.instructions)
                if isinstance(inst, mybir.InstLoadActFuncSet)]
        if not idxs:
            continue
        first = blk.instructions[idxs[0]]
        assert first.sync_info is None
        first.act_func_set_id = 6
        for i in reversed(idxs[1:]):
            assert blk.instructions[i].sync_info is None
            del blk.instructions[i]


def _prep_weights(proj_w, proj_b, sim_alpha, sim_beta, proj2_w, proj2_b):
    BF = ml_dtypes.bfloat16
    pwT = np.ascontiguousarray(proj_w.T)                       # [384, 1920]
    pwt = np.ascontiguousarray(
        pwT.reshape(3, 128, 1920).transpose(1, 0, 2)).astype(BF)
    G = (0, 1, 3)
    pbn = np.empty((48, 24), np.float32)
    for t in range(24):
        g, h = divmod(t, 8)
        o0 = G[g] * 384 + 48 * h
        pbn[:, t] = proj_b[o0:o0 + 48]
    pbv = np.concatenate([proj_b[768:1152], proj_b[1536:1920]])[None, :]
    p2b = np.ascontiguousarray(proj2_b[None, :]).astype(np.float32)
    alph = np.tile(sim_alpha, 8)[None, :].astype(np.float32)
    beta = np.tile(sim_beta, 8)[None, :].astype(np.float32)
    p2wT = np.ascontiguousarray(proj2_w.T)                     # [384, 384]
    p2w = np.ascontiguousarray(
        p2wT.reshape(8, 48, 384).transpose(1, 0, 2)).astype(BF)
    mup = _upsample_matrix().astype(BF)
    return {"pwt": pwt, "pbn": pbn,
            "pbv": np.ascontiguousarray(pbv).astype(np.float32),
            "p2b": p2b, "alph": alph, "beta": beta,
            "p2w": p2w, "mup": mup}


def kernel(x, proj_w, proj_b, sim_alpha, sim_beta, proj2_w, proj2_b):
    x = np.asarray(x, np.float32)
    proj_w = np.asarray(proj_w, np.float32)
    proj_b = np.asarray(proj_b, np.float32)
    sim_alpha = np.asarray(sim_alpha, np.float32)
    sim_beta = np.asarray(sim_beta, np.float32)
    proj2_w = np.asarray(proj2_w, np.float32)
    proj2_b = np.asarray(proj2_b, np.float32)

    if "nc" not in _CACHE:
        _CACHE["nc"] = build_nc()
    nc = _CACHE["nc"]

    w = _prep_weights(proj_w, proj_b, sim_alpha, sim_beta, proj2_w, proj2_b)
    B = x.shape[0]
    xr = x.reshape(8, B // 8, 384, 3136).astype(ml_dtypes.bfloat16)
    in_maps = [dict(w, x=np.ascontiguousarray(xr[c])) for c in range(8)]

    res = run_bass_kernel_spmd(nc, in_maps, core_ids=list(range(8)))
    out = np.concatenate([np.asarray(r["y"], dtype=np.float32)
                          for r in res.results], axis=0)
    return out.reshape(16, 384, 56, 56)


if __name__ == "__main__":
    rng = np.random.default_rng(0)
    inputs = {
        "x": rng.standard_normal((16, 384, 56, 56), dtype=np.float32),
        "proj_w": rng.standard_normal((1920, 384), dtype=np.float32) * 384 ** -0.5,
        "proj_b": np.zeros(1920, np.float32),
        "sim_alpha": np.ones(8, np.float32),
        "sim_beta": np.zeros(8, np.float32),
        "proj2_w": rng.standard_normal((384, 384), dtype=np.float32) * 384 ** -0.5,
        "proj2_b": np.zeros(384, np.float32),
    }
    out = kernel(**inputs)
    print("kernel ran, output", out.shape, out.dtype, float(np.abs(out).max()))


# revision 23
# speedup vs baseline: 1.3093x; 1.3093x over previous
"""Trainium2 Bass kernel for nn_EnsembleClustering_62646392979777.

Strategy (validated against the reference by a numpy prototype):
  * The full-resolution projection y = proj(x) is only ever consumed through
    spatial average-pools (7x7 agents, 2x2 clusters), and pooling commutes
    with the 1x1 conv.  So pool x first (56x56 -> 14x14 -> {7x7, 2x2}),
    then project the 53 pooled positions instead of 3136.  This removes
    ~98% of the FLOPs; the kernel becomes HBM-bound on reading x and
    writing the output.
  * proj2 and the bilinear upsample also commute: run proj2 on the 7x7
    grid, then upsample as a dense [49 x 3136] matmul (exact linear op).
  * Data-parallel over batch: 16 batches -> 8 cores x 2.
  * The 8 cores share one chip's HBM, so bytes are the roofline: x and y
    travel as bf16 (host casts, ~0.5% l2 error, gate is 2e-2).  Matmuls
    run bf16 (PE 1 cycle/row vs 4 for fp32); softmax/norm math stays fp32.
  * Pooling is restructured as packed bf16 tensor_tensor adds (DVE 2x
    mode) for the h-direction plus strided pair-adds for w, instead of
    monolithic reduces (reduces get no DVE fast modes).
  * All scalar-engine activations use one table set (exp/ln/square/copy)
    to avoid 1.3us act-table reloads; 1/sqrt is exp(-0.5*ln(s)).
  * Softmax max-subtraction is dropped: pooled scores are O(0.03), and
    |similarity| <= |alpha|+|beta| ~ 1, so exp never overflows fp32.

  * Emission is software-pipelined: the two batches' attention chains are
    round-robin interleaved (generator zip) together with the previous
    rep's upsample chunks, so every engine's in-order instruction stream
    has ready work while any one chain waits on a dependency.

Layout rules honored throughout: compute-engine partition bases are
32-aligned (BIR-verifier enforced); matmul operand bases in {0,32,64};
per-slice work is stacked along the free dimension.

Measured on 8 axon-tunneled TRN2 NeuronCores (hardware For_i-loop NEFF,
reps-delta timing): relative error 6.7e-3; steady-state ~63-64 us per
iteration (2 batches), vs ~"90+ us" for the same math without the
bf16/act-table/software-pipelining work. The 8 cores share one chip's
HBM (~2.5 TB/s aggregate measured), so the bf16 I/O halving was the
single biggest lever.
"""
import sys
import numpy as np

sys.path.insert(0, "/opt/trn_rl_repo")

import ml_dtypes  # noqa: E402
import concourse.bass as bass  # noqa: E402
import concourse.tile as tile  # noqa: E402
from concourse import bacc, mybir  # noqa: E402
from concourse.bass_utils import run_bass_kernel_spmd  # noqa: E402
from concourse.masks import make_identity  # noqa: E402

F32 = mybir.dt.float32
BF16 = mybir.dt.bfloat16
AX = mybir.AxisListType
AF = mybir.ActivationFunctionType
OP = mybir.AluOpType

EPS = 1e-6
INV_SQRT_C = float(1.0 / np.sqrt(np.float32(48.0)))

_CACHE = {}


def _upsample_matrix():
    # jax.image.resize 'linear' 7->56 upsample: half-pixel centers, edge clamp
    U = np.zeros((56, 7), dtype=np.float64)
    for o in range(56):
        src = (o + 0.5) / 8.0 - 0.5
        i0 = int(np.floor(src))
        t = src - i0
        U[o, min(max(i0, 0), 6)] += 1.0 - t
        U[o, min(max(i0 + 1, 0), 6)] += t
    U = U.astype(np.float32)
    return np.einsum("Oi,Pj->ijOP", U, U).reshape(49, 3136).copy()


def build_nc(reps=1, stage="full", loop=False):
    # stage: "full" | "pool" (DMA-in + pooling only) | "noup" (skip upsample)
    # loop=True wraps the body in a hardware For_i loop (reps iterations of
    # a 2-rep body) so steady-state timing NEFFs compile in O(1) and run
    # long enough to swamp dispatch jitter.
    nc = bacc.Bacc("TRN2", target_bir_lowering=False, debug=False,
                   enable_asserts=False)

    x_d = nc.dram_tensor("x", [2, 384, 3136], BF16, kind="ExternalInput").ap()
    pwt_d = nc.dram_tensor("pwt", [128, 3, 1920], BF16, kind="ExternalInput").ap()
    pbn_d = nc.dram_tensor("pbn", [48, 24], F32, kind="ExternalInput").ap()
    pwtb_d = nc.dram_tensor("pwtb", [1, 1920], BF16, kind="ExternalInput").ap()
    p2wb_d = nc.dram_tensor("p2wb", [1, 384], BF16, kind="ExternalInput").ap()
    al_d = nc.dram_tensor("alph", [1, 64], F32, kind="ExternalInput").ap()
    be_d = nc.dram_tensor("beta", [1, 64], F32, kind="ExternalInput").ap()
    p2w_d = nc.dram_tensor("p2w", [48, 8, 384], BF16, kind="ExternalInput").ap()
    mup_d = nc.dram_tensor("mup", [49, 3136], BF16, kind="ExternalInput").ap()
    y_d = nc.dram_tensor("y", [2, 384, 3136], BF16, kind="ExternalOutput").ap()

    with tile.TileContext(nc) as tc:
        with tc.tile_pool(name="w", bufs=1) as wp, \
             tc.tile_pool(name="xin", bufs=3) as xin, \
             tc.tile_pool(name="pool", bufs=2) as pp, \
             tc.tile_pool(name="st", bufs=2) as st, \
             tc.tile_pool(name="out", bufs=2) as outp, \
             tc.tile_pool(name="ps", bufs=2, space="PSUM") as ps:

            # ---------------- constants & weights ----------------
            identB = wp.tile([128, 128], BF16, tag="identB")
            make_identity(nc, identB[:])
            ones_c = wp.tile([49, 1], F32, tag="ones_c")
            nc.vector.memset(ones_c[:], 1.0)
            ones_cb = wp.tile([49, 1], BF16, tag="ones_cb")
            nc.vector.memset(ones_cb[:], 1.0)
            ones_r = wp.tile([1, 768], F32, tag="ones_r")
            nc.vector.memset(ones_r[:], 1.0)

            PWT = wp.tile([128, 3, 1920], BF16, tag="pwt")
            nc.sync.dma_start(PWT[:], pwt_d)
            P2W = wp.tile([48, 8, 384], BF16, tag="p2w")
            nc.sync.dma_start(P2W[:], p2w_d)
            MUP = wp.tile([49, 3136], BF16, tag="mup")
            nc.sync.dma_start(MUP[:], mup_d)
            PBN = wp.tile([48, 24], F32, tag="pbn")
            nc.sync.dma_start(PBN[:], pbn_d)
            PWTB = wp.tile([128, 1920], BF16, tag="pwtb")
            nc.vector.memset(PWTB[:], 0.0)
            nc.sync.dma_start(PWTB[0:1, :], pwtb_d)
            P2WB = wp.tile([48, 384], BF16, tag="p2wb")
            nc.vector.memset(P2WB[:], 0.0)
            nc.sync.dma_start(P2WB[0:1, :], p2wb_d)
            XPb = wp.tile([128, 53], BF16, tag="xpb")
            nc.vector.memset(XPb[:], 0.0)
            nc.vector.memset(XPb[0:1, :], 1.0)
            Zfb = wp.tile([48, 49], BF16, tag="zfb")
            nc.vector.memset(Zfb[:], 0.0)
            nc.vector.memset(Zfb[0:1, :], 1.0)
            AL1 = wp.tile([1, 64], F32, tag="al1")
            nc.sync.dma_start(AL1[:], al_d)
            BE1 = wp.tile([1, 64], F32, tag="be1")
            nc.sync.dma_start(BE1[:], be_d)

            # broadcast alpha/beta/bias rows across partitions via K=1 matmuls
            ALB = wp.tile([49, 64], F32, tag="alb")
            BEB = wp.tile([49, 64], F32, tag="beb")
            for src, dst in ((AL1, ALB), (BE1, BEB)):
                pt = ps.tile([49, 64], F32, tag="C")
                nc.tensor.matmul(pt[:], ones_r[:, :49], src[:], start=True, stop=True)
                nc.vector.tensor_copy(dst[:], pt[:])

            # ---------------- per-batch pipeline ----------------
            # reps>1 re-emits the body for steady-state benchmarking
            def _pool_gen(rep_bi, pout):
                bi = rep_bi % 2
                # ---- Stage A: load & pool x (bf16) ----
                # h-pools as packed pair-adds (DVE 2x mode), w-pools strided.
                XP = st.tile([128, 3, 53], BF16, tag=f"xp{bi}")
                X3 = xin.tile([128, 3, 3136], BF16, tag="x")
                nc.sync.dma_start(
                    X3[:], x_d[bi].rearrange("(j p) n -> p j n", p=128))
                for j in range(3):
                    X = X3[:, j, :]
                    Xv = X.rearrange("p (a two b) -> p a two b", two=2, b=56)
                    A = pp.tile([128, 28, 56], BF16, tag="pa")
                    nc.vector.tensor_tensor(A[:], Xv[:, :, 0, :], Xv[:, :, 1, :], OP.add)
                    Av = A[:].rearrange("p (a two) b -> p a two b", two=2)
                    Bt = pp.tile([128, 14, 56], BF16, tag="pb")
                    nc.vector.tensor_tensor(Bt[:], Av[:, :, 0, :], Av[:, :, 1, :], OP.add)
                    Bv = Bt[:].rearrange("p a (c two) -> p a c two", two=2)
                    # w-pools on gpsimd (SBUF-only bf16): frees the DVE for
                    # the PSUM->SBUF copies it alone (with ACT) can do.
                    D = pp.tile([128, 14, 28], BF16, tag="pd")
                    nc.gpsimd.tensor_tensor(D[:], Bv[:, :, :, 0], Bv[:, :, :, 1], OP.add)
                    Dv = D[:].rearrange("p a (c two) -> p a c two", two=2)
                    C14 = pp.tile([128, 14, 14], BF16, tag="pc")
                    nc.gpsimd.tensor_tensor(C14[:], Dv[:, :, :, 0], Dv[:, :, :, 1], OP.add)
                    Cv = C14[:].rearrange("p (g two) w -> p g two w", two=2)
                    E = pp.tile([128, 7, 14], BF16, tag="pe")
                    nc.vector.tensor_tensor(E[:], Cv[:, :, 0, :], Cv[:, :, 1, :], OP.add)
                    Ev = E[:].rearrange("p g (f two) -> p g f two", two=2)
                    F = pp.tile([128, 7, 7], BF16, tag="pf")
                    nc.vector.tensor_tensor(F[:], Ev[:, :, :, 0], Ev[:, :, :, 1], OP.add)
                    nc.vector.tensor_scalar_mul(
                        XP[:, j, 0:49], F[:].rearrange("p a b -> p (a b)"), 1.0 / 64.0)
                    P2t = pp.tile([128, 2, 2], F32, tag="pg")
                    nc.vector.reduce_sum(
                        P2t[:],
                        C14[:].rearrange("p (oh hi) (ow wi) -> p oh ow hi wi",
                                         oh=2, ow=2),
                        axis=AX.XY)
                    nc.vector.tensor_scalar_mul(
                        XP[:, j, 49:53], P2t[:].rearrange("p a b -> p (a b)"),
                        1.0 / 784.0)
                    yield True

                if stage == "pool":
                    ri = (rep_bi // 2) % 19
                    nc.sync.dma_start(
                        y_d[bi, 0:128, 159 * ri:159 * (ri + 1)],
                        XP[:].rearrange("p a b -> p (a b)"))
                    pout[rep_bi] = None
                    return
                pout[rep_bi] = (bi, XP)

            def _mid_gen(rep_bi, bi, XP, out):
                # ---- Stage B1: natural-layout projection (groups p, k0, k1) ----
                # chunk t = g*8 + h ; o-range = G[g]*384 + 48h,  G = [0, 1, 3]
                # k-groups (g=1,2) first: the score matmuls depend only on them.
                Ysb = st.tile([48, 24, 53], BF16, tag=f"ysb{bi}")
                for gi, (g, G) in enumerate(((1, 1), (2, 3), (0, 0))):
                    pt = ps.tile([48, 8, 53], F32, tag="A")
                    for h in range(8):
                        o0 = G * 384 + 48 * h
                        for j in range(3):
                            nc.tensor.matmul(pt[:, h, :], PWT[:, j, o0:o0 + 48],
                                             XP[:, j, :], start=(j == 0), stop=(j == 2))
                    nc.vector.tensor_tensor(
                        Ysb[:, 8 * g:8 * (g + 1), :], pt[:],
                        PBN[:, 8 * g:8 * (g + 1), None].to_broadcast((48, 8, 53)),
                        OP.add)
                    yield True

                # ---- Stage B2: transposed projection of v-groups ----
                # V7T[n, i*384 + hc] = y7[(2+2i)*384 + hc, n];  VCt likewise
                V7T = st.tile([49, 768], BF16, tag=f"v7t{bi}")
                VCt = st.tile([4, 768], BF16, tag=f"vct{bi}")
                for i in range(2):
                    o0 = (2 + 2 * i) * 384
                    pt = ps.tile([49, 384], F32, tag="B")
                    for j in range(3):
                        nc.tensor.matmul(pt[:], XP[:, j, 0:49], PWT[:, j, o0:o0 + 384],
                                         start=(j == 0), stop=False)
                    nc.tensor.matmul(pt[:], XPb[:, 0:49], PWTB[:, o0:o0 + 384],
                                     start=False, stop=True)
                    nc.scalar.copy(V7T[:, 384 * i:384 * (i + 1)], pt[:])
                    pt2 = ps.tile([4, 384], F32, tag="C")
                    for j in range(3):
                        nc.tensor.matmul(pt2[:], XP[:, j, 49:53], PWT[:, j, o0:o0 + 384],
                                         start=(j == 0), stop=False)
                    nc.tensor.matmul(pt2[:], XPb[:, 49:53], PWTB[:, o0:o0 + 384],
                                     start=False, stop=True)
                    nc.scalar.copy(VCt[:, 384 * i:384 * (i + 1)], pt2[:])
                    yield True

                # p-token norms early: independent of the attention chain
                # rp = (sum_c p^2)^-1/2 via exp(-0.5 * ln s): stays in one
                # activation-table set (set 6: exp/ln/square/copy).
                SQP = st.tile([48, 8, 49], BF16, tag=f"sqp{bi}")
                nc.vector.tensor_tensor(SQP[:], Ysb[:, 0:8, 0:49], Ysb[:, 0:8, 0:49],
                                        OP.mult)
                ptn = ps.tile([49, 8], F32, tag="C")
                for h in range(8):
                    nc.tensor.matmul(ptn[:, h:h + 1], SQP[:, h, :], ones_cb[:48, :],
                                     start=True, stop=True)
                RP = st.tile([49, 8], F32, tag=f"rp{bi}")
                nc.scalar.activation(RP[:], ptn[:], AF.Ln)
                nc.scalar.activation(RP[:], RP[:], AF.Exp, scale=-0.5)
                yield True

                # ---- Stage C: attention / clustering (8 slices, free-stacked) ----
                # module-0 scores -> all 8 slices in one PSUM bank; exp reads
                # PSUM directly with the 1/sqrt(c) scale folded in (no max
                # subtraction needed: |scores| ~ 0.03).
                pt0 = ps.tile([4, 8, 49], F32, tag="C")
                pt1 = ps.tile([49, 8, 4], F32, tag="B")
                for h in range(8):
                    nc.tensor.matmul(pt0[:, h, :], Ysb[:, 8 + h, 49:53],
                                     Ysb[:, 8 + h, 0:49], start=True, stop=True)
                    nc.tensor.matmul(pt1[:, h, :], Ysb[:, 16 + h, 0:49],
                                     Ysb[:, 16 + h, 49:53], start=True, stop=True)

                # softmax0 over n (module 0)
                E0 = st.tile([4, 8, 49], F32, tag=f"e0{bi}")
                nc.scalar.activation(E0[:], pt0[:], AF.Exp, scale=INV_SQRT_C)
                SM0 = st.tile([4, 8], F32, tag=f"sm0{bi}")
                nc.vector.reduce_sum(SM0[:], E0[:], axis=AX.X)
                nc.vector.reciprocal(SM0[:], SM0[:])
                yield True
                A0 = st.tile([4, 8, 49], BF16, tag=f"a0{bi}")
                nc.vector.tensor_tensor(A0[:], E0[:],
                                        SM0[:, :, None].to_broadcast((4, 8, 49)), OP.mult)

                # softmax1 over clusters (module 1)
                E1 = st.tile([49, 8, 4], F32, tag=f"e1{bi}")
                nc.scalar.activation(E1[:], pt1[:], AF.Exp, scale=INV_SQRT_C)
                SM1 = st.tile([49, 8], F32, tag=f"sm1{bi}")
                nc.vector.reduce_sum(SM1[:], E1[:], axis=AX.X)
                nc.vector.reciprocal(SM1[:], SM1[:])
                yield True
                A1T = st.tile([49, 8, 4], BF16, tag=f"a1t{bi}")
                nc.vector.tensor_tensor(A1T[:], E1[:],
                                        SM1[:, :, None].to_broadcast((49, 8, 4)), OP.mult)

                # transpose module-0 attention: A0 [4,49] slices -> A0T [49,(8,4)]
                ptA = ps.tile([49, 8, 4], BF16, tag="C")
                for h in range(8):
                    nc.tensor.transpose(ptA[:, h, :], A0[:, h, :], identB[:4, :4])
                A0T = st.tile([49, 8, 4], BF16, tag=f"a0t{bi}")
                nc.vector.tensor_copy(A0T[:], ptA[:])

                # fuzzy-membership normalizer: 1/(sum_n memb + eps), PE-broadcast
                yield True
                ptd = ps.tile([1, 32], F32, tag="C")
                nc.tensor.matmul(ptd[:], ones_cb[:], A1T[:].rearrange("p a b -> p (a b)"),
                                 start=True, stop=True)
                DE = st.tile([1, 32], F32, tag=f"de{bi}")
                nc.vector.tensor_scalar_add(DE[:], ptd[:], EPS)
                nc.vector.reciprocal(DE[:], DE[:])
                ptb = ps.tile([49, 32], F32, tag="B")
                nc.tensor.matmul(ptb[:], ones_r[:, :49], DE[:], start=True, stop=True)
                A1N = st.tile([49, 8, 4], BF16, tag=f"a1n{bi}")
                nc.vector.tensor_tensor(A1N[:].rearrange("p a b -> p (a b)"),
                                        A1T[:].rearrange("p a b -> p (a b)"),
                                        ptb[:], OP.mult)

                yield True
                # agg = attn @ v (+ vc), stacked [4, (2, 8, 48)]
                AGGf = st.tile([4, 2, 8, 48], BF16, tag=f"aggf{bi}")
                for i in range(2):
                    AT = A0T if i == 0 else A1N
                    ptg = ps.tile([4, 8, 48], F32, tag="C" if i == 0 else "B")
                    for h in range(8):
                        nc.tensor.matmul(ptg[:, h, :], AT[:, h, :],
                                         V7T[:, 384 * i + 48 * h:384 * i + 48 * (h + 1)],
                                         start=True, stop=True)
                    nc.vector.tensor_tensor(AGGf[:, i, :, :], ptg[:],
                                      VCt[:, 384 * i:384 * (i + 1)].rearrange(
                                          "p (a b) -> p a b", b=48), OP.add)

                # agg row norms -> normalized AGGN (rsqrt via exp(-0.5 ln))
                yield True
                SQ = st.tile([4, 768], BF16, tag=f"sq{bi}")
                nc.vector.tensor_tensor(SQ[:], AGGf[:].rearrange("p a b c -> p (a b c)"),
                                        AGGf[:].rearrange("p a b c -> p (a b c)"),
                                        OP.mult)
                SS = st.tile([4, 16], F32, tag=f"ss{bi}")
                nc.vector.reduce_sum(SS[:], SQ[:].rearrange("p (g c) -> p g c", c=48),
                                     axis=AX.X)
                nc.scalar.activation(SS[:], SS[:], AF.Ln)
                nc.scalar.activation(SS[:], SS[:], AF.Exp, scale=-0.5)
                AGGN = st.tile([4, 2, 8, 48], BF16, tag=f"aggn{bi}")
                nc.vector.tensor_tensor(
                    AGGN[:].rearrange("p a b c -> p (a b) c"),
                    AGGf[:].rearrange("p a b c -> p (a b) c"),
                    SS[:, :, None].to_broadcast((4, 16, 48)), OP.mult)

                yield True
                # transpose AGGN slices -> AGGNT [48, (8 slices, 8 m)]
                ptT = ps.tile([48, 8, 8], BF16, tag="C")
                for i in range(2):
                    for h in range(8):
                        nc.tensor.transpose(ptT[:, h, 4 * i:4 * (i + 1)],
                                            AGGN[:, i, h, :], identB[:4, :4])
                AGGNT = st.tile([48, 8, 8], BF16, tag=f"aggnt{bi}")
                nc.scalar.copy(AGGNT[:], ptT[:])

                # sim^T [49, (8 slices, 8 m)] = (p^T @ aggn^T) * rp, alpha/beta
                ptm = ps.tile([49, 64], F32, tag="C")
                for h in range(8):
                    nc.tensor.matmul(ptm[:, 8 * h:8 * (h + 1)], Ysb[:, h, 0:49],
                                     AGGNT[:, h, :], start=True, stop=True)
                yield True
                SIMT = st.tile([49, 8, 8], F32, tag=f"simt{bi}")
                nc.vector.tensor_tensor(SIMT[:], ptm[:].rearrange("p (a b) -> p a b", b=8),
                                        RP[:, :, None].to_broadcast((49, 8, 8)), OP.mult)
                nc.vector.tensor_tensor(SIMT[:].rearrange("p a b -> p (a b)"),
                                        SIMT[:].rearrange("p a b -> p (a b)"),
                                        ALB[:], OP.mult)
                nc.vector.tensor_tensor(SIMT[:].rearrange("p a b -> p (a b)"),
                                        SIMT[:].rearrange("p a b -> p (a b)"),
                                        BEB[:], OP.add)

                # assignment softmax over the 8 clusters (inner free dim);
                # |similarity| <= 1 so no max subtraction.
                yield True
                EX = st.tile([49, 8, 8], F32, tag=f"ex{bi}")
                nc.scalar.activation(EX[:], SIMT[:], AF.Exp)
                SMS = st.tile([49, 8], F32, tag=f"sms{bi}")
                nc.vector.reduce_sum(SMS[:], EX[:], axis=AX.X)
                nc.vector.reciprocal(SMS[:], SMS[:])
                ASGT = st.tile([49, 8, 8], BF16, tag=f"asgt{bi}")
                nc.vector.tensor_tensor(ASGT[:], EX[:],
                                        SMS[:, :, None].to_broadcast((49, 8, 8)), OP.mult)

                yield True
                # transpose assignment to m-on-partitions (two 4-row banks)
                # inner dim padded to 50: bf16 PSUM sub-writes must stay
                # 4-byte aligned (49*2 = 98B offsets are not)
                ptS0 = ps.tile([4, 8, 50], BF16, tag="C")
                ptS1 = ps.tile([4, 8, 50], BF16, tag="B")
                for h in range(8):
                    nc.tensor.transpose(ptS0[:, h, 0:49], ASGT[:, h, 0:4], identB[:49, :49])
                    nc.tensor.transpose(ptS1[:, h, 0:49], ASGT[:, h, 4:8], identB[:49, :49])
                ASG0 = st.tile([4, 8, 49], BF16, tag=f"asg0{bi}")
                ASG1 = st.tile([4, 8, 49], BF16, tag=f"asg1{bi}")
                nc.scalar.copy(ASG0[:], ptS0[:, :, 0:49])
                nc.vector.tensor_copy(ASG1[:], ptS1[:, :, 0:49])

                yield True
                # out_low per slice: z [48, 49] = agg^T @ assignment
                ptZ = ps.tile([48, 8, 49], F32, tag="A")
                for h in range(8):
                    nc.tensor.matmul(ptZ[:, h, :], AGGf[:, 0, h, :], ASG0[:, h, :],
                                     start=True, stop=False)
                    nc.tensor.matmul(ptZ[:, h, :], AGGf[:, 1, h, :], ASG1[:, h, :],
                                     start=False, stop=True)
                Zf = st.tile([48, 8, 49], BF16, tag=f"zf{bi}")
                nc.scalar.copy(Zf[:], ptZ[:])

                yield True
                # proj2 on the 7x7 grid, transposed layout: [49, 384] out
                ptP = ps.tile([49, 384], F32, tag="B")
                for h in range(8):
                    nc.tensor.matmul(ptP[:], Zf[:, h, :], P2W[:, h, :],
                                     start=(h == 0), stop=False)
                nc.tensor.matmul(ptP[:], Zfb[:], P2WB[:], start=False, stop=True)
                Z2T = st.tile([49, 384], BF16, tag=f"z2t{bi}")
                nc.scalar.copy(Z2T[:], ptP[:])

                if stage == "noup":
                    ri = (rep_bi // 2) % 8
                    nc.sync.dma_start(y_d[bi, 0:49, 384 * ri:384 * (ri + 1)], Z2T[:])
                    out[bi] = None
                    return
                out[bi] = Z2T

            def _back_gen(bi, Z2T):
                # upsample: out[128, 3136] = z2T^T @ MUP, tile N by 448.
                # Emitted as a generator so the NEXT batch's front half can
                # interleave these ready-to-run chunks into every engine's
                # in-order stream at its dependency stall points.
                for oi in range(3):
                    OUT = outp.tile([128, 3136], BF16, tag="out")
                    for nt in range(7):
                        pt = ps.tile([128, 448], F32, tag="U")
                        nc.tensor.matmul(pt[:],
                                         Z2T[:, 128 * oi:128 * (oi + 1)],
                                         MUP[:, 448 * nt:448 * (nt + 1)],
                                         start=True, stop=True)
                        eng = nc.vector if nt % 2 == 1 else nc.scalar
                        if eng is nc.vector:
                            nc.vector.tensor_copy(OUT[:, 448 * nt:448 * (nt + 1)], pt[:])
                        else:
                            nc.scalar.copy(OUT[:, 448 * nt:448 * (nt + 1)], pt[:])
                        if nt % 2 == 1:
                            yield True
                    nc.gpsimd.dma_start(y_d[bi, 128 * oi:128 * (oi + 1), :], OUT[:])
                    yield True

            def _zip(gens):
                # round-robin the generators: each engine's in-order stream
                # alternates between independent chains, so a stalled chain
                # never head-of-line-blocks ready work from the others.
                gens = [g for g in gens if g is not None]
                while gens:
                    nxt = []
                    for g in gens:
                        if next(g, None) is not None:
                            nxt.append(g)
                    gens = nxt

            def _body(n_reps):
                # 3-deep software pipeline: rep k's attention chains are
                # round-robined with rep k-1's upsample and rep k+1's pooling,
                # so each engine's in-order stream always has ready work.
                pout = {}
                backs = []
                for k in range(n_reps):
                    _zip([_pool_gen(2 * k, pout), _pool_gen(2 * k + 1, pout)])
                    outz = {}
                    mids = []
                    for rb in (2 * k, 2 * k + 1):
                        p = pout.pop(rb, None)
                        if p is not None:
                            mids.append(_mid_gen(rb, p[0], p[1], outz))
                    _zip(mids + backs)
                    backs = [_back_gen(b, z) for b, z in sorted(outz.items())
                             if z is not None]
                _zip(backs)

            if loop:
                U = 8  # reps per hardware-loop iteration (amortizes back-edge)
                assert reps % U == 0
                with tc.For_i(0, reps // U, 1, staggered_reset=True,
                              hint_engines=(mybir.EngineType.PE,)):
                    _body(U)
            else:
                _body(reps)

    nc.compile()
    _fix_act_table_loads(nc)
    return nc


def _fix_act_table_loads(nc):
    # Every activation used here (exp/ln/square/copy/identity) lives in
    # act-table set 6 (natural_log_exp_and_others), but the auto-inserted
    # loads pick the first set per function (5 for ln, 0 for exp), so the
    # 1.3us table load thrashes before nearly every Ln/Exp pair.  Rewrite
    # to a single set-6 load per block.
    for blk in nc.m.functions[0].blocks:
        idxs = [i for i, inst in enumerate(blk.instructions)
                if isinstance(inst, mybir.InstLoadActFuncSet)]
        if not idxs:
            continue
        first = blk.instructions[idxs[0]]
        assert first.sync_info is None
        first.act_func_set_id = 6
        for i in reversed(idxs[1:]):
            assert blk.instructions[i].sync_info is None
            del blk.instructions[i]


def _prep_weights(proj_w, proj_b, sim_alpha, sim_beta, proj2_w, proj2_b):
    BF = ml_dtypes.bfloat16
    pwT = np.ascontiguousarray(proj_w.T)                       # [384, 1920]
    pwt = np.ascontiguousarray(
        pwT.reshape(3, 128, 1920).transpose(1, 0, 2)).astype(BF)
    G = (0, 1, 3)
    pbn = np.empty((48, 24), np.float32)
    for t in range(24):
        g, h = divmod(t, 8)
        o0 = G[g] * 384 + 48 * h
        pbn[:, t] = proj_b[o0:o0 + 48]
    pwtb = np.ascontiguousarray(proj_b[None, :]).astype(BF)
    p2wb = np.ascontiguousarray(proj2_b[None, :]).astype(BF)
    alph = np.tile(sim_alpha, 8)[None, :].astype(np.float32)
    beta = np.tile(sim_beta, 8)[None, :].astype(np.float32)
    p2wT = np.ascontiguousarray(proj2_w.T)                     # [384, 384]
    p2w = np.ascontiguousarray(
        p2wT.reshape(8, 48, 384).transpose(1, 0, 2)).astype(BF)
    mup = _upsample_matrix().astype(BF)
    return {"pwt": pwt, "pbn": pbn,
            "pwtb": pwtb, "p2wb": p2wb, "alph": alph, "beta": beta,
            "p2w": p2w, "mup": mup}


def kernel(x, proj_w, proj_b, sim_alpha, sim_beta, proj2_w, proj2_b):
    x = np.asarray(x, np.float32)
    proj_w = np.asarray(proj_w, np.float32)
    proj_b = np.asarray(proj_b, np.float32)
    sim_alpha = np.asarray(sim_alpha, np.float32)
    sim_beta = np.asarray(sim_beta, np.float32)
    proj2_w = np.asarray(proj2_w, np.float32)
    proj2_b = np.asarray(proj2_b, np.float32)

    if "nc" not in _CACHE:
        _CACHE["nc"] = build_nc()
    nc = _CACHE["nc"]

    w = _prep_weights(proj_w, proj_b, sim_alpha, sim_beta, proj2_w, proj2_b)
    B = x.shape[0]
    xr = x.reshape(8, B // 8, 384, 3136).astype(ml_dtypes.bfloat16)
    in_maps = [dict(w, x=np.ascontiguousarray(xr[c])) for c in range(8)]

    res = run_bass_kernel_spmd(nc, in_maps, core_ids=list(range(8)))
    out = np.concatenate([np.asarray(r["y"], dtype=np.float32)
                          for r in res.results], axis=0)
    return out.reshape(16, 384, 56, 56)


if __name__ == "__main__":
    rng = np.random.default_rng(0)
    inputs = {
        "x": rng.standard_normal((16, 384, 56, 56), dtype=np.float32),
        "proj_w": rng.standard_normal((1920, 384), dtype=np.float32) * 384 ** -0.5,
        "proj_b": np.zeros(1920, np.float32),
        "sim_alpha": np.ones(8, np.float32),
        "sim_beta": np.zeros(8, np.float32),
        "proj2_w": rng.standard_normal((384, 384), dtype=np.float32) * 384 ** -0.5,
        "proj2_b": np.zeros(384, np.float32),
    }
    out = kernel(**inputs)
    print("kernel ran, output", out.shape, out.dtype, float(np.abs(out).max()))


# revision 26
# speedup vs baseline: 1.3286x; 1.0147x over previous
"""Trainium2 Bass kernel for nn_EnsembleClustering_62646392979777.

Strategy (validated against the reference by a numpy prototype):
  * The full-resolution projection y = proj(x) is only ever consumed through
    spatial average-pools (7x7 agents, 2x2 clusters), and pooling commutes
    with the 1x1 conv.  So pool x first (56x56 -> 14x14 -> {7x7, 2x2}),
    then project the 53 pooled positions instead of 3136.  This removes
    ~98% of the FLOPs; the kernel becomes HBM-bound on reading x and
    writing the output.
  * proj2 and the bilinear upsample also commute: run proj2 on the 7x7
    grid, then upsample as a dense [49 x 3136] matmul (exact linear op).
  * Data-parallel over batch: 16 batches -> 8 cores x 2.
  * The 8 cores share one chip's HBM, so bytes are the roofline: x and y
    travel as bf16 (host casts, ~0.5% l2 error, gate is 2e-2).  Matmuls
    run bf16 (PE 1 cycle/row vs 4 for fp32); softmax/norm math stays fp32.
  * Pooling is restructured as packed bf16 tensor_tensor adds (DVE 2x
    mode) for the h-direction plus strided pair-adds for w, instead of
    monolithic reduces (reduces get no DVE fast modes).
  * All scalar-engine activations use one table set (exp/ln/square/copy)
    to avoid 1.3us act-table reloads; 1/sqrt is exp(-0.5*ln(s)).
  * Softmax max-subtraction is dropped: pooled scores are O(0.03), and
    |similarity| <= |alpha|+|beta| ~ 1, so exp never overflows fp32.

  * Emission is software-pipelined: the two batches' attention chains are
    round-robin interleaved (generator zip) together with the previous
    rep's upsample chunks, so every engine's in-order instruction stream
    has ready work while any one chain waits on a dependency.

Layout rules honored throughout: compute-engine partition bases are
32-aligned (BIR-verifier enforced); matmul operand bases in {0,32,64};
per-slice work is stacked along the free dimension.

Measured on 8 axon-tunneled TRN2 NeuronCores (hardware For_i-loop NEFF,
reps-delta timing): relative error 6.7e-3; steady-state ~63-64 us per
iteration (2 batches), vs ~"90+ us" for the same math without the
bf16/act-table/software-pipelining work. The 8 cores share one chip's
HBM (~2.5 TB/s aggregate measured), so the bf16 I/O halving was the
single biggest lever.
"""
import sys
import numpy as np

sys.path.insert(0, "/opt/trn_rl_repo")

import ml_dtypes  # noqa: E402
import concourse.bass as bass  # noqa: E402
import concourse.tile as tile  # noqa: E402
from concourse import bacc, mybir  # noqa: E402
from concourse.bass_utils import run_bass_kernel_spmd  # noqa: E402
from concourse.masks import make_identity  # noqa: E402

F32 = mybir.dt.float32
BF16 = mybir.dt.bfloat16
AX = mybir.AxisListType
AF = mybir.ActivationFunctionType
OP = mybir.AluOpType

EPS = 1e-6
INV_SQRT_C = float(1.0 / np.sqrt(np.float32(48.0)))

_CACHE = {}


def _upsample_matrix():
    # jax.image.resize 'linear' 7->56 upsample: half-pixel centers, edge clamp
    U = np.zeros((56, 7), dtype=np.float64)
    for o in range(56):
        src = (o + 0.5) / 8.0 - 0.5
        i0 = int(np.floor(src))
        t = src - i0
        U[o, min(max(i0, 0), 6)] += 1.0 - t
        U[o, min(max(i0 + 1, 0), 6)] += t
    U = U.astype(np.float32)
    return np.einsum("Oi,Pj->ijOP", U, U).reshape(49, 3136).copy()


def build_nc(reps=1, stage="full", loop=False):
    # stage: "full" | "pool" (DMA-in + pooling only) | "noup" (skip upsample)
    # loop=True wraps the body in a hardware For_i loop (reps iterations of
    # a 2-rep body) so steady-state timing NEFFs compile in O(1) and run
    # long enough to swamp dispatch jitter.
    nc = bacc.Bacc("TRN2", target_bir_lowering=False, debug=False,
                   enable_asserts=False)

    x_d = nc.dram_tensor("x", [2, 384, 3136], BF16, kind="ExternalInput").ap()
    pwt_d = nc.dram_tensor("pwt", [128, 3, 1920], BF16, kind="ExternalInput").ap()
    pbn_d = nc.dram_tensor("pbn", [48, 24], F32, kind="ExternalInput").ap()
    pwtb_d = nc.dram_tensor("pwtb", [1, 1920], BF16, kind="ExternalInput").ap()
    p2wb_d = nc.dram_tensor("p2wb", [1, 384], BF16, kind="ExternalInput").ap()
    al_d = nc.dram_tensor("alph", [1, 64], F32, kind="ExternalInput").ap()
    be_d = nc.dram_tensor("beta", [1, 64], F32, kind="ExternalInput").ap()
    p2w_d = nc.dram_tensor("p2w", [48, 8, 384], BF16, kind="ExternalInput").ap()
    mup_d = nc.dram_tensor("mup", [49, 3136], BF16, kind="ExternalInput").ap()
    y_d = nc.dram_tensor("y", [2, 384, 3136], BF16, kind="ExternalOutput").ap()

    with tile.TileContext(nc) as tc:
        with tc.tile_pool(name="w", bufs=1) as wp, \
             tc.tile_pool(name="xin", bufs=3) as xin, \
             tc.tile_pool(name="pool", bufs=2) as pp, \
             tc.tile_pool(name="st", bufs=2) as st, \
             tc.tile_pool(name="out", bufs=2) as outp, \
             tc.tile_pool(name="ps", bufs=2, space="PSUM") as ps:

            # ---------------- constants & weights ----------------
            identB = wp.tile([128, 128], BF16, tag="identB")
            make_identity(nc, identB[:])
            ones_c = wp.tile([49, 1], F32, tag="ones_c")
            nc.vector.memset(ones_c[:], 1.0)
            ones_cb = wp.tile([49, 1], BF16, tag="ones_cb")
            nc.vector.memset(ones_cb[:], 1.0)
            ones_r = wp.tile([1, 768], F32, tag="ones_r")
            nc.vector.memset(ones_r[:], 1.0)

            PWT = wp.tile([128, 3, 1920], BF16, tag="pwt")
            nc.sync.dma_start(PWT[:], pwt_d)
            P2W = wp.tile([48, 8, 384], BF16, tag="p2w")
            nc.sync.dma_start(P2W[:], p2w_d)
            MUP = wp.tile([49, 3136], BF16, tag="mup")
            nc.sync.dma_start(MUP[:], mup_d)
            PBN = wp.tile([48, 24], F32, tag="pbn")
            nc.sync.dma_start(PBN[:], pbn_d)
            PWTB = wp.tile([128, 1920], BF16, tag="pwtb")
            nc.vector.memset(PWTB[:], 0.0)
            nc.sync.dma_start(PWTB[0:1, :], pwtb_d)
            P2WB = wp.tile([48, 384], BF16, tag="p2wb")
            nc.vector.memset(P2WB[:], 0.0)
            nc.sync.dma_start(P2WB[0:1, :], p2wb_d)
            XPb = wp.tile([128, 53], BF16, tag="xpb")
            nc.vector.memset(XPb[:], 0.0)
            nc.vector.memset(XPb[0:1, :], 1.0)
            Zfb = wp.tile([48, 49], BF16, tag="zfb")
            nc.vector.memset(Zfb[:], 0.0)
            nc.vector.memset(Zfb[0:1, :], 1.0)
            AL1 = wp.tile([1, 64], F32, tag="al1")
            nc.sync.dma_start(AL1[:], al_d)
            BE1 = wp.tile([1, 64], F32, tag="be1")
            nc.sync.dma_start(BE1[:], be_d)

            # broadcast alpha/beta/bias rows across partitions via K=1 matmuls
            ALB = wp.tile([49, 64], F32, tag="alb")
            BEB = wp.tile([49, 64], F32, tag="beb")
            for src, dst in ((AL1, ALB), (BE1, BEB)):
                pt = ps.tile([49, 64], F32, tag="C", bufs=3)
                nc.tensor.matmul(pt[:], ones_r[:, :49], src[:], start=True, stop=True)
                nc.vector.tensor_copy(dst[:], pt[:])

            # ---------------- per-batch pipeline ----------------
            # reps>1 re-emits the body for steady-state benchmarking
            def _pool_gen(rep_bi, pout):
                bi = rep_bi % 2
                # ---- Stage A: load & pool x (bf16) ----
                # h-pools as packed pair-adds (DVE 2x mode), w-pools strided.
                XP = st.tile([128, 3, 53], BF16, tag=f"xp{bi}")
                X3 = xin.tile([128, 3, 3136], BF16, tag="x")
                nc.sync.dma_start(
                    X3[:], x_d[bi].rearrange("(j p) n -> p j n", p=128))
                for j in range(3):
                    X = X3[:, j, :]
                    Xv = X.rearrange("p (a two b) -> p a two b", two=2, b=56)
                    A = pp.tile([128, 28, 56], BF16, tag="pa")
                    nc.vector.tensor_tensor(A[:], Xv[:, :, 0, :], Xv[:, :, 1, :], OP.add)
                    Av = A[:].rearrange("p (a two) b -> p a two b", two=2)
                    Bt = pp.tile([128, 14, 56], BF16, tag="pb")
                    nc.vector.tensor_tensor(Bt[:], Av[:, :, 0, :], Av[:, :, 1, :], OP.add)
                    Bv = Bt[:].rearrange("p a (c two) -> p a c two", two=2)
                    # w-pools on gpsimd (SBUF-only bf16): frees the DVE for
                    # the PSUM->SBUF copies it alone (with ACT) can do.
                    D = pp.tile([128, 14, 28], BF16, tag="pd")
                    nc.gpsimd.tensor_tensor(D[:], Bv[:, :, :, 0], Bv[:, :, :, 1], OP.add)
                    Dv = D[:].rearrange("p a (c two) -> p a c two", two=2)
                    C14 = pp.tile([128, 14, 14], BF16, tag="pc")
                    nc.gpsimd.tensor_tensor(C14[:], Dv[:, :, :, 0], Dv[:, :, :, 1], OP.add)
                    Cv = C14[:].rearrange("p (g two) w -> p g two w", two=2)
                    E = pp.tile([128, 7, 14], BF16, tag="pe")
                    nc.vector.tensor_tensor(E[:], Cv[:, :, 0, :], Cv[:, :, 1, :], OP.add)
                    Ev = E[:].rearrange("p g (f two) -> p g f two", two=2)
                    F = pp.tile([128, 7, 7], BF16, tag="pf")
                    nc.vector.tensor_tensor(F[:], Ev[:, :, :, 0], Ev[:, :, :, 1], OP.add)
                    nc.vector.tensor_scalar_mul(
                        XP[:, j, 0:49], F[:].rearrange("p a b -> p (a b)"), 1.0 / 64.0)
                    P2t = pp.tile([128, 2, 2], F32, tag="pg")
                    nc.vector.reduce_sum(
                        P2t[:],
                        C14[:].rearrange("p (oh hi) (ow wi) -> p oh ow hi wi",
                                         oh=2, ow=2),
                        axis=AX.XY)
                    nc.vector.tensor_scalar_mul(
                        XP[:, j, 49:53], P2t[:].rearrange("p a b -> p (a b)"),
                        1.0 / 784.0)
                    yield True

                if stage == "pool":
                    ri = (rep_bi // 2) % 19
                    nc.sync.dma_start(
                        y_d[bi, 0:128, 159 * ri:159 * (ri + 1)],
                        XP[:].rearrange("p a b -> p (a b)"))
                    pout[rep_bi] = None
                    return
                pout[rep_bi] = (bi, XP)

            def _mid_gen(rep_bi, bi, XP, out):
                # ---- Stage B1: natural-layout projection (groups p, k0, k1) ----
                # chunk t = g*8 + h ; o-range = G[g]*384 + 48h,  G = [0, 1, 3]
                # k-groups (g=1,2) first: the score matmuls depend only on them.
                Ysb = st.tile([48, 24, 53], BF16, tag=f"ysb{bi}")
                for gi, (g, G) in enumerate(((1, 1), (2, 3), (0, 0))):
                    pt = ps.tile([48, 8, 53], F32, tag="A", bufs=1)
                    for h in range(8):
                        o0 = G * 384 + 48 * h
                        for j in range(3):
                            nc.tensor.matmul(pt[:, h, :], PWT[:, j, o0:o0 + 48],
                                             XP[:, j, :], start=(j == 0), stop=(j == 2))
                    nc.vector.tensor_tensor(
                        Ysb[:, 8 * g:8 * (g + 1), :], pt[:],
                        PBN[:, 8 * g:8 * (g + 1), None].to_broadcast((48, 8, 53)),
                        OP.add)
                    yield True

                # ---- Stage B2: transposed projection of v-groups ----
                # V7T[n, i*384 + hc] = y7[(2+2i)*384 + hc, n];  VCt likewise
                V7T = st.tile([49, 768], BF16, tag=f"v7t{bi}")
                VCt = st.tile([4, 768], BF16, tag=f"vct{bi}")
                for i in range(2):
                    o0 = (2 + 2 * i) * 384
                    pt = ps.tile([49, 384], F32, tag="B")
                    for j in range(3):
                        nc.tensor.matmul(pt[:], XP[:, j, 0:49], PWT[:, j, o0:o0 + 384],
                                         start=(j == 0), stop=False)
                    nc.tensor.matmul(pt[:], XPb[:, 0:49], PWTB[:, o0:o0 + 384],
                                     start=False, stop=True)
                    nc.scalar.copy(V7T[:, 384 * i:384 * (i + 1)], pt[:])
                    pt2 = ps.tile([4, 384], F32, tag="C", bufs=3)
                    for j in range(3):
                        nc.tensor.matmul(pt2[:], XP[:, j, 49:53], PWT[:, j, o0:o0 + 384],
                                         start=(j == 0), stop=False)
                    nc.tensor.matmul(pt2[:], XPb[:, 49:53], PWTB[:, o0:o0 + 384],
                                     start=False, stop=True)
                    nc.scalar.copy(VCt[:, 384 * i:384 * (i + 1)], pt2[:])
                    yield True

                # p-token norms early: independent of the attention chain
                # rp = (sum_c p^2)^-1/2 via exp(-0.5 * ln s): stays in one
                # activation-table set (set 6: exp/ln/square/copy).
                SQP = st.tile([48, 8, 49], BF16, tag=f"sqp{bi}")
                nc.vector.tensor_tensor(SQP[:], Ysb[:, 0:8, 0:49], Ysb[:, 0:8, 0:49],
                                        OP.mult)
                ptn = ps.tile([49, 8], F32, tag="C", bufs=3)
                for h in range(8):
                    nc.tensor.matmul(ptn[:, h:h + 1], SQP[:, h, :], ones_cb[:48, :],
                                     start=True, stop=True)
                RP = st.tile([49, 8], F32, tag=f"rp{bi}")
                nc.scalar.activation(RP[:], ptn[:], AF.Ln)
                nc.scalar.activation(RP[:], RP[:], AF.Exp, scale=-0.5)
                yield True

                # ---- Stage C: attention / clustering (8 slices, free-stacked) ----
                # module-0 scores -> all 8 slices in one PSUM bank; exp reads
                # PSUM directly with the 1/sqrt(c) scale folded in (no max
                # subtraction needed: |scores| ~ 0.03).
                pt0 = ps.tile([4, 8, 49], F32, tag="C", bufs=3)
                pt1 = ps.tile([49, 8, 4], F32, tag="B")
                for h in range(8):
                    nc.tensor.matmul(pt0[:, h, :], Ysb[:, 8 + h, 49:53],
                                     Ysb[:, 8 + h, 0:49], start=True, stop=True)
                    nc.tensor.matmul(pt1[:, h, :], Ysb[:, 16 + h, 0:49],
                                     Ysb[:, 16 + h, 49:53], start=True, stop=True)

                # softmax0 over n (module 0)
                E0 = st.tile([4, 8, 49], F32, tag=f"e0{bi}")
                nc.scalar.activation(E0[:], pt0[:], AF.Exp, scale=INV_SQRT_C)
                SM0 = st.tile([4, 8], F32, tag=f"sm0{bi}")
                nc.vector.reduce_sum(SM0[:], E0[:], axis=AX.X)
                nc.vector.reciprocal(SM0[:], SM0[:])
                yield True
                A0 = st.tile([4, 8, 49], BF16, tag=f"a0{bi}")
                nc.vector.tensor_tensor(A0[:], E0[:],
                                        SM0[:, :, None].to_broadcast((4, 8, 49)), OP.mult)

                # softmax1 over clusters (module 1)
                E1 = st.tile([49, 8, 4], F32, tag=f"e1{bi}")
                nc.scalar.activation(E1[:], pt1[:], AF.Exp, scale=INV_SQRT_C)
                SM1 = st.tile([49, 8], F32, tag=f"sm1{bi}")
                nc.vector.reduce_sum(SM1[:], E1[:], axis=AX.X)
                nc.vector.reciprocal(SM1[:], SM1[:])
                yield True
                A1T = st.tile([49, 8, 4], BF16, tag=f"a1t{bi}")
                nc.vector.tensor_tensor(A1T[:], E1[:],
                                        SM1[:, :, None].to_broadcast((49, 8, 4)), OP.mult)

                # transpose module-0 attention: A0 [4,49] slices -> A0T [49,(8,4)]
                ptA = ps.tile([49, 8, 4], BF16, tag="C", bufs=3)
                for h in range(8):
                    nc.tensor.transpose(ptA[:, h, :], A0[:, h, :], identB[:4, :4])
                A0T = st.tile([49, 8, 4], BF16, tag=f"a0t{bi}")
                nc.vector.tensor_copy(A0T[:], ptA[:])

                # fuzzy-membership normalizer: 1/(sum_n memb + eps), PE-broadcast
                yield True
                ptd = ps.tile([1, 32], F32, tag="C", bufs=3)
                nc.tensor.matmul(ptd[:], ones_cb[:], A1T[:].rearrange("p a b -> p (a b)"),
                                 start=True, stop=True)
                DE = st.tile([1, 32], F32, tag=f"de{bi}")
                nc.vector.tensor_scalar_add(DE[:], ptd[:], EPS)
                nc.vector.reciprocal(DE[:], DE[:])
                ptb = ps.tile([49, 32], F32, tag="B")
                nc.tensor.matmul(ptb[:], ones_r[:, :49], DE[:], start=True, stop=True)
                A1N = st.tile([49, 8, 4], BF16, tag=f"a1n{bi}")
                nc.vector.tensor_tensor(A1N[:].rearrange("p a b -> p (a b)"),
                                        A1T[:].rearrange("p a b -> p (a b)"),
                                        ptb[:], OP.mult)

                yield True
                # agg = attn @ v (+ vc), stacked [4, (2, 8, 48)]
                AGGf = st.tile([4, 2, 8, 48], BF16, tag=f"aggf{bi}")
                for i in range(2):
                    AT = A0T if i == 0 else A1N
                    ptg = ps.tile([4, 8, 48], F32, tag="C" if i == 0 else "B",
                                  bufs=3 if i == 0 else None)
                    for h in range(8):
                        nc.tensor.matmul(ptg[:, h, :], AT[:, h, :],
                                         V7T[:, 384 * i + 48 * h:384 * i + 48 * (h + 1)],
                                         start=True, stop=True)
                    nc.vector.tensor_tensor(AGGf[:, i, :, :], ptg[:],
                                      VCt[:, 384 * i:384 * (i + 1)].rearrange(
                                          "p (a b) -> p a b", b=48), OP.add)

                # agg row norms -> normalized AGGN (rsqrt via exp(-0.5 ln))
                yield True
                SQ = st.tile([4, 768], BF16, tag=f"sq{bi}")
                nc.vector.tensor_tensor(SQ[:], AGGf[:].rearrange("p a b c -> p (a b c)"),
                                        AGGf[:].rearrange("p a b c -> p (a b c)"),
                                        OP.mult)
                SS = st.tile([4, 16], F32, tag=f"ss{bi}")
                nc.vector.reduce_sum(SS[:], SQ[:].rearrange("p (g c) -> p g c", c=48),
                                     axis=AX.X)
                nc.scalar.activation(SS[:], SS[:], AF.Ln)
                nc.scalar.activation(SS[:], SS[:], AF.Exp, scale=-0.5)
                AGGN = st.tile([4, 2, 8, 48], BF16, tag=f"aggn{bi}")
                nc.vector.tensor_tensor(
                    AGGN[:].rearrange("p a b c -> p (a b) c"),
                    AGGf[:].rearrange("p a b c -> p (a b) c"),
                    SS[:, :, None].to_broadcast((4, 16, 48)), OP.mult)

                yield True
                # transpose AGGN slices -> AGGNT [48, (8 slices, 8 m)]
                ptT = ps.tile([48, 8, 8], BF16, tag="C", bufs=3)
                for i in range(2):
                    for h in range(8):
                        nc.tensor.transpose(ptT[:, h, 4 * i:4 * (i + 1)],
                                            AGGN[:, i, h, :], identB[:4, :4])
                AGGNT = st.tile([48, 8, 8], BF16, tag=f"aggnt{bi}")
                nc.scalar.copy(AGGNT[:], ptT[:])

                # sim^T [49, (8 slices, 8 m)] = (p^T @ aggn^T) * rp, alpha/beta
                ptm = ps.tile([49, 64], F32, tag="C", bufs=3)
                for h in range(8):
                    nc.tensor.matmul(ptm[:, 8 * h:8 * (h + 1)], Ysb[:, h, 0:49],
                                     AGGNT[:, h, :], start=True, stop=True)
                yield True
                SIMT = st.tile([49, 8, 8], F32, tag=f"simt{bi}")
                nc.vector.tensor_tensor(SIMT[:], ptm[:].rearrange("p (a b) -> p a b", b=8),
                                        RP[:, :, None].to_broadcast((49, 8, 8)), OP.mult)
                nc.vector.tensor_tensor(SIMT[:].rearrange("p a b -> p (a b)"),
                                        SIMT[:].rearrange("p a b -> p (a b)"),
                                        ALB[:], OP.mult)
                nc.vector.tensor_tensor(SIMT[:].rearrange("p a b -> p (a b)"),
                                        SIMT[:].rearrange("p a b -> p (a b)"),
                                        BEB[:], OP.add)

                # assignment softmax over the 8 clusters (inner free dim);
                # |similarity| <= 1 so no max subtraction.
                yield True
                EX = st.tile([49, 8, 8], F32, tag=f"ex{bi}")
                nc.scalar.activation(EX[:], SIMT[:], AF.Exp)
                SMS = st.tile([49, 8], F32, tag=f"sms{bi}")
                nc.vector.reduce_sum(SMS[:], EX[:], axis=AX.X)
                nc.vector.reciprocal(SMS[:], SMS[:])
                ASGT = st.tile([49, 8, 8], BF16, tag=f"asgt{bi}")
                nc.vector.tensor_tensor(ASGT[:], EX[:],
                                        SMS[:, :, None].to_broadcast((49, 8, 8)), OP.mult)

                yield True
                # transpose assignment to m-on-partitions (two 4-row banks)
                # inner dim padded to 50: bf16 PSUM sub-writes must stay
                # 4-byte aligned (49*2 = 98B offsets are not)
                ptS0 = ps.tile([4, 8, 50], BF16, tag="C", bufs=3)
                ptS1 = ps.tile([4, 8, 50], BF16, tag="B")
                for h in range(8):
                    nc.tensor.transpose(ptS0[:, h, 0:49], ASGT[:, h, 0:4], identB[:49, :49])
                    nc.tensor.transpose(ptS1[:, h, 0:49], ASGT[:, h, 4:8], identB[:49, :49])
                ASG0 = st.tile([4, 8, 49], BF16, tag=f"asg0{bi}")
                ASG1 = st.tile([4, 8, 49], BF16, tag=f"asg1{bi}")
                nc.scalar.copy(ASG0[:], ptS0[:, :, 0:49])
                nc.vector.tensor_copy(ASG1[:], ptS1[:, :, 0:49])

                yield True
                # out_low per slice: z [48, 49] = agg^T @ assignment
                ptZ = ps.tile([48, 8, 49], F32, tag="A", bufs=1)
                for h in range(8):
                    nc.tensor.matmul(ptZ[:, h, :], AGGf[:, 0, h, :], ASG0[:, h, :],
                                     start=True, stop=False)
                    nc.tensor.matmul(ptZ[:, h, :], AGGf[:, 1, h, :], ASG1[:, h, :],
                                     start=False, stop=True)
                Zf = st.tile([48, 8, 49], BF16, tag=f"zf{bi}")
                nc.scalar.copy(Zf[:], ptZ[:])

                yield True
                # proj2 on the 7x7 grid, transposed layout: [49, 384] out
                ptP = ps.tile([49, 384], F32, tag="B")
                for h in range(8):
                    nc.tensor.matmul(ptP[:], Zf[:, h, :], P2W[:, h, :],
                                     start=(h == 0), stop=False)
                nc.tensor.matmul(ptP[:], Zfb[:], P2WB[:], start=False, stop=True)
                Z2T = st.tile([49, 384], BF16, tag=f"z2t{bi}")
                nc.scalar.copy(Z2T[:], ptP[:])

                if stage == "noup":
                    ri = (rep_bi // 2) % 8
                    nc.sync.dma_start(y_d[bi, 0:49, 384 * ri:384 * (ri + 1)], Z2T[:])
                    out[bi] = None
                    return
                out[bi] = Z2T

            def _back_gen(bi, Z2T):
                # upsample: out[128, 3136] = z2T^T @ MUP, tile N by 448.
                # Emitted as a generator so the NEXT batch's front half can
                # interleave these ready-to-run chunks into every engine's
                # in-order stream at its dependency stall points.
                for oi in range(3):
                    OUT = outp.tile([128, 3136], BF16, tag="out")
                    for nt in range(7):
                        pt = ps.tile([128, 448], F32, tag="U")
                        nc.tensor.matmul(pt[:],
                                         Z2T[:, 128 * oi:128 * (oi + 1)],
                                         MUP[:, 448 * nt:448 * (nt + 1)],
                                         start=True, stop=True)
                        eng = nc.vector if nt % 2 == 1 else nc.scalar
                        if eng is nc.vector:
                            nc.vector.tensor_copy(OUT[:, 448 * nt:448 * (nt + 1)], pt[:])
                        else:
                            nc.scalar.copy(OUT[:, 448 * nt:448 * (nt + 1)], pt[:])
                        if nt % 2 == 1:
                            yield True
                    nc.gpsimd.dma_start(y_d[bi, 128 * oi:128 * (oi + 1), :], OUT[:])
                    yield True

            def _zip(gens):
                # round-robin the generators: each engine's in-order stream
                # alternates between independent chains, so a stalled chain
                # never head-of-line-blocks ready work from the others.
                gens = [g for g in gens if g is not None]
                while gens:
                    nxt = []
                    for g in gens:
                        if next(g, None) is not None:
                            nxt.append(g)
                    gens = nxt

            def _body(n_reps):
                # 3-deep software pipeline: rep k's attention chains are
                # round-robined with rep k-1's upsample and rep k+1's pooling,
                # so each engine's in-order stream always has ready work.
                pout = {}
                backs = []
                for k in range(n_reps):
                    _zip([_pool_gen(2 * k, pout), _pool_gen(2 * k + 1, pout)])
                    outz = {}
                    mids = []
                    for rb in (2 * k, 2 * k + 1):
                        p = pout.pop(rb, None)
                        if p is not None:
                            mids.append(_mid_gen(rb, p[0], p[1], outz))
                    _zip(mids + backs)
                    backs = [_back_gen(b, z) for b, z in sorted(outz.items())
                             if z is not None]
                _zip(backs)

            if loop:
                U = 16  # reps per hardware-loop iteration (amortizes back-edge)
                assert reps % U == 0
                with tc.For_i(0, reps // U, 1, staggered_reset=True,
                              hint_engines=(mybir.EngineType.PE,)):
                    _body(U)
            else:
                _body(reps)

    nc.compile()
    _fix_act_table_loads(nc)
    return nc


def _fix_act_table_loads(nc):
    # Every activation used here (exp/ln/square/copy/identity) lives in
    # act-table set 6 (natural_log_exp_and_others), but the auto-inserted
    # loads pick the first set per function (5 for ln, 0 for exp), so the
    # 1.3us table load thrashes before nearly every Ln/Exp pair.  Rewrite
    # to a single set-6 load per block.
    for blk in nc.m.functions[0].blocks:
        idxs = [i for i, inst in enumerate(blk.instructions)
                if isinstance(inst, mybir.InstLoadActFuncSet)]
        if not idxs:
            continue
        first = blk.instructions[idxs[0]]
        assert first.sync_info is None
        first.act_func_set_id = 6
        for i in reversed(idxs[1:]):
            assert blk.instructions[i].sync_info is None
            del blk.instructions[i]


def _prep_weights(proj_w, proj_b, sim_alpha, sim_beta, proj2_w, proj2_b):
    BF = ml_dtypes.bfloat16
    pwT = np.ascontiguousarray(proj_w.T)                       # [384, 1920]
    pwt = np.ascontiguousarray(
        pwT.reshape(3, 128, 1920).transpose(1, 0, 2)).astype(BF)
    G = (0, 1, 3)
    pbn = np.empty((48, 24), np.float32)
    for t in range(24):
        g, h = divmod(t, 8)
        o0 = G[g] * 384 + 48 * h
        pbn[:, t] = proj_b[o0:o0 + 48]
    pwtb = np.ascontiguousarray(proj_b[None, :]).astype(BF)
    p2wb = np.ascontiguousarray(proj2_b[None, :]).astype(BF)
    alph = np.tile(sim_alpha, 8)[None, :].astype(np.float32)
    beta = np.tile(sim_beta, 8)[None, :].astype(np.float32)
    p2wT = np.ascontiguousarray(proj2_w.T)                     # [384, 384]
    p2w = np.ascontiguousarray(
        p2wT.reshape(8, 48, 384).transpose(1, 0, 2)).astype(BF)
    mup = _upsample_matrix().astype(BF)
    return {"pwt": pwt, "pbn": pbn,
            "pwtb": pwtb, "p2wb": p2wb, "alph": alph, "beta": beta,
            "p2w": p2w, "mup": mup}


def kernel(x, proj_w, proj_b, sim_alpha, sim_beta, proj2_w, proj2_b):
    x = np.asarray(x, np.float32)
    proj_w = np.asarray(proj_w, np.float32)
    proj_b = np.asarray(proj_b, np.float32)
    sim_alpha = np.asarray(sim_alpha, np.float32)
    sim_beta = np.asarray(sim_beta, np.float32)
    proj2_w = np.asarray(proj2_w, np.float32)
    proj2_b = np.asarray(proj2_b, np.float32)

    if "nc" not in _CACHE:
        _CACHE["nc"] = build_nc()
    nc = _CACHE["nc"]

    w = _prep_weights(proj_w, proj_b, sim_alpha, sim_beta, proj2_w, proj2_b)
    B = x.shape[0]
    xr = x.reshape(8, B // 8, 384, 3136).astype(ml_dtypes.bfloat16)
    in_maps = [dict(w, x=np.ascontiguousarray(xr[c])) for c in range(8)]

    res = run_bass_kernel_spmd(nc, in_maps, core_ids=list(range(8)))
    out = np.concatenate([np.asarray(r["y"], dtype=np.float32)
                          for r in res.results], axis=0)
    return out.reshape(16, 384, 56, 56)


if __name__ == "__main__":
    rng = np.random.default_rng(0)
    inputs = {
        "x": rng.standard_normal((16, 384, 56, 56), dtype=np.float32),
        "proj_w": rng.standard_normal((1920, 384), dtype=np.float32) * 384 ** -0.5,
        "proj_b": np.zeros(1920, np.float32),
        "sim_alpha": np.ones(8, np.float32),
        "sim_beta": np.zeros(8, np.float32),
        "proj2_w": rng.standard_normal((384, 384), dtype=np.float32) * 384 ** -0.5,
        "proj2_b": np.zeros(384, np.float32),
    }
    out = kernel(**inputs)
    print("kernel ran, output", out.shape, out.dtype, float(np.abs(out).max()))
